# revision 32
# baseline (speedup 1.0000x reference)
"""Trainium2 Bass kernel for nn_PhysicsSparseMoE.

Strategy (data-parallel over B, one batch per core):

With TOPK=1 the reference's token-level routing collapses: every token in
batch b is dispatched entirely (weight exactly 1.0) to the single expert
e_b selected by the video-level router, and the dispatch one-hot column of
the fusion input contributes fus_w1[:, 768+e_b] as a constant bias.  The
expert scan over all 17 experts is therefore mathematically identical to
running just expert e_b per batch (adding we=0 terms is exact in fp32).

Host side: the tiny time-aware summary + router (O(B*T*C)) picks e_b per
batch and computes the scalar balance loss; expert weights are gathered
per batch and shipped to the owning core (this *is* the sharding step).

Device side (per core, batch b, 1024 tokens, feature-on-partition layout):
    h1  = gelu(W1 @ xT + b1)            [3072, 1024]   (fc1 of expert e_b)
    y   = W2 @ h1 + b2                  [768, 1024]    (= "aggregated".T)
    h2  = gelu(F1 @ y + b1f)            [768, 1024]    (fus_w1[:, :768], bias
                                                        includes dispatch col)
    out = F2 @ h2 + (y + b2f)           [768, 1024]
All matmuls run as float32r (1 cycle/row at N=512) accumulating in fp32
PSUM; gelu is the exact (erf) ACT table function.  Tokens are processed in
two 512-column chunks so the y accumulator fits in 6 PSUM banks; fc1/fc2
weights stream from HBM once per chunk, fusion weights and xT stay
resident in SBUF.  DMA emission order tracks first-use order so the
critical path (first token chunk of x + first fc1 column block) is not
queued behind later data.
"""

import numpy as np
from contextlib import ExitStack

import concourse.bass as bass
import concourse.tile as tile
from concourse.tile import add_dep_helper
import concourse.mybir as mybir
from concourse import bacc
from concourse.bass_utils import run_bass_kernel_spmd

# Problem constants (hardcoded per task contract).
B = 8
N = 1024
C = 768
H = 3072
E = 17
KEY_TOP_M = 3
KEY_ALPHA = 0.5

P = 128
CC = C // P           # 6 feature chunks
HC = H // P           # 24 hidden chunks
TCH = 512             # token chunk (moving free dim)
NCH = N // TCH        # 2 token chunks
JG = 4                # hidden chunks per weight-stream group
NJG = HC // JG        # 6 groups
NB = HC + 3 * CC      # packed bias columns (b1 | b2 | b1f | b2f)

F32 = mybir.dt.float32
F32R = mybir.dt.float32r
GELU = mybir.ActivationFunctionType.Gelu
IDENT = mybir.ActivationFunctionType.Identity

_CACHED_NC = None


def _build_nc():
    nc = bacc.Bacc()

    xt_d = nc.declare_dram_parameter("xt", [C, N], F32R, isOutput=False)
    w1t_d = nc.declare_dram_parameter("w1t", [C, H], F32R, isOutput=False)
    w2t_d = nc.declare_dram_parameter("w2t", [H, C], F32R, isOutput=False)
    fw1t_d = nc.declare_dram_parameter("fw1t", [C, C], F32R, isOutput=False)
    fw2t_d = nc.declare_dram_parameter("fw2t", [C, C], F32R, isOutput=False)
    bias_d = nc.declare_dram_parameter("bias", [P, NB], F32, isOutput=False)
    out_d = nc.declare_dram_parameter("outT", [C, N], F32, isOutput=True)

    with tile.TileContext(nc) as tc, ExitStack() as ctx:
        consts = ctx.enter_context(tc.tile_pool(name="consts", bufs=1))
        w1_pool = ctx.enter_context(tc.tile_pool(name="w1s", bufs=3))
        w2_pool = ctx.enter_context(tc.tile_pool(name="w2s", bufs=3))
        h1_pool = ctx.enter_context(tc.tile_pool(name="h1", bufs=4))
        y_pool = ctx.enter_context(tc.tile_pool(name="y", bufs=2))
        h2_pool = ctx.enter_context(tc.tile_pool(name="h2", bufs=2))
        o_pool = ctx.enter_context(tc.tile_pool(name="o", bufs=1))
        acc_ps = ctx.enter_context(tc.tile_pool(name="accps", bufs=6, space="PSUM"))
        mm_ps = ctx.enter_context(tc.tile_pool(name="mmps", bufs=2, space="PSUM"))

        # Resident tensors.  xt is loaded per token chunk (the second chunk's
        # columns are only needed ~90us in); fusion weights load during the
        # first chunk's FFN phase.
        xt = consts.tile([P, CC, N], F32R)
        nc.sync.dma_start(
            xt[:, :, 0:TCH],
            xt_d[:, 0:TCH].rearrange("(i p) n -> p i n", p=P))
        biases = consts.tile([P, NB], F32)
        b1 = biases[:, 0:HC]
        b2 = biases[:, HC:HC + CC]
        b1f = biases[:, HC + CC:HC + 2 * CC]
        b2f = biases[:, HC + 2 * CC:HC + 3 * CC]
        fw1t = consts.tile([P, CC, C], F32R)
        fw2t = consts.tile([P, CC, C], F32R)

        # Warm the PE clock (HAM un-throttles after ~3.4us of activity) with
        # throwaway matmuls on scratch buffers while the first DMAs land.
        warm_in = consts.tile([P, TCH], F32)
        nc.gpsimd.memset(warm_in[:], 0.0)
        warm_ps = mm_ps.tile([P, TCH], F32, tag="mm", name="warm_ps")
        for _ in range(4):
            nc.tensor.matmul(warm_ps[:], warm_in[:, 0:P], warm_in[:],
                             start=True, stop=True)

        out_dmas_c0 = []
        for t in range(NCH):
            tok = bass.ts(t, TCH)
            last = t == NCH - 1

            # ---- expert FFN: h1 = gelu(W1 x + b1); y_psum += W2 h1 ----
            y_ps = [acc_ps.tile([P, TCH], F32, tag="acc", name=f"y_ps_{t}_{k}")
                    for k in range(CC)]
            for jg in range(NJG):
                w1g = w1_pool.tile([P, CC, JG * P], F32R)
                w1_src = w1t_d[:, jg * JG * P:(jg + 1) * JG * P]
                if t == 0 and jg == 0:
                    # split so the j=0 column block (and with it the first
                    # matmul) lands as early as possible
                    nc.sync.dma_start(
                        w1g[:, :, 0:P],
                        w1_src[:, 0:P].rearrange("(i p) h -> p i h", p=P))
                    nc.sync.dma_start(biases[:], bias_d[:])
                    nc.sync.dma_start(
                        w1g[:, :, P:JG * P],
                        w1_src[:, P:JG * P].rearrange("(i p) h -> p i h", p=P))
                else:
                    w1_anchor = nc.sync.dma_start(
                        w1g[:], w1_src.rearrange("(i p) h -> p i h", p=P))
                w2g = w2_pool.tile([P, JG, C], F32R)
                nc.sync.dma_start(
                    w2g[:],
                    w2t_d[jg * JG * P:(jg + 1) * JG * P, :].rearrange(
                        "(j p) c -> p j c", p=P),
                )
                for jj in range(JG):
                    j = jg * JG + jj
                    h_ps = mm_ps.tile([P, TCH], F32, tag="mm")
                    for i in range(CC):
                        nc.tensor.matmul(
                            h_ps[:], w1g[:, i, jj * P:(jj + 1) * P],
                            xt[:, i, tok],
                            start=(i == 0), stop=(i == CC - 1),
                        )
                    h1 = h1_pool.tile([P, TCH], F32R)
                    nc.scalar.activation(h1[:], h_ps[:], GELU,
                                         bias=b1[:, j:j + 1])
                    for k in range(CC):
                        nc.tensor.matmul(
                            y_ps[k][:], w2g[:, jj, k * P:(k + 1) * P], h1[:],
                            start=(j == 0), stop=(j == HC - 1),
                        )

            if t == 0:
                nc.sync.dma_start(
                    xt[:, :, TCH:N],
                    xt_d[:, TCH:N].rearrange("(i p) n -> p i n", p=P))
                nc.sync.dma_start(
                    fw1t[:], fw1t_d.rearrange("(i p) m -> p i m", p=P))
                nc.sync.dma_start(
                    fw2t[:], fw2t_d.rearrange("(i p) m -> p i m", p=P))

            # ---- evict y with fc2 bias (split across ACT and DVE so the
            # fusion phase can start sooner) ----
            y = y_pool.tile([P, CC, TCH], F32R)
            for k in range(CC):
                if k % 2 == 0:
                    nc.scalar.activation(y[:, k, :], y_ps[k][:], IDENT,
                                         bias=b2[:, k:k + 1])
                else:
                    nc.vector.tensor_scalar_add(y[:, k, :], y_ps[k][:],
                                                b2[:, k:k + 1])

            # ---- fusion layer 1: h2 = gelu(F1 y + b1f) ----
            h2 = h2_pool.tile([P, CC, TCH], F32R)
            for m in range(CC):
                h2_ps = mm_ps.tile([P, TCH], F32, tag="mm")
                for k in range(CC):
                    nc.tensor.matmul(
                        h2_ps[:], fw1t[:, k, m * P:(m + 1) * P], y[:, k, :],
                        start=(k == 0), stop=(k == CC - 1),
                    )
                nc.scalar.activation(h2[:, m, :], h2_ps[:], GELU,
                                     bias=b1f[:, m:m + 1])

            # after fusion layer 1 has consumed y, fold the final bias into
            # the residual in place (DVE), so the output stage is one add
            for k in range(CC):
                nc.vector.tensor_scalar_add(y[:, k, :], y[:, k, :],
                                            b2f[:, k:k + 1])

            # ---- fusion layer 2 + residual: out = F2 h2 + (y + b2f) ----
            # o_ps shares the "mm" psum slots mid-kernel (so the 6 "acc"
            # banks free up for the next chunk's y accumulation); on the last
            # chunk it uses the idle "acc" banks to avoid slot contention.
            o_sb = o_pool.tile([P, CC, TCH], F32)
            out_dmas = out_dmas_c0 if t == 0 else None
            for k in range(CC):
                o_ps = mm_ps.tile([P, TCH], F32, tag="mm",
                                  name=f"o_ps_{t}_{k}") if not last else \
                    acc_ps.tile([P, TCH], F32, tag="acc",
                                name=f"o_ps_{t}_{k}")
                for m in range(CC):
                    nc.tensor.matmul(
                        o_ps[:], fw2t[:, m, k * P:(k + 1) * P], h2[:, m, :],
                        start=(m == 0), stop=(m == CC - 1),
                    )
                if last and k == CC - 1:
                    # split the final add+writeback so the drain tail overlaps
                    hf = TCH // 2
                    for h in range(2):
                        sl = bass.ts(h, hf)
                        nc.vector.tensor_add(o_sb[:, k, sl], o_ps[:, sl],
                                             y[:, k, sl])
                        nc.sync.dma_start(
                            out_d[k * P:(k + 1) * P,
                                  t * TCH + h * hf:t * TCH + (h + 1) * hf],
                            o_sb[:, k, sl])
                else:
                    nc.vector.tensor_add(o_sb[:, k, :], o_ps[:], y[:, k, :])
                    d = nc.sync.dma_start(out_d[k * P:(k + 1) * P, tok],
                                          o_sb[:, k, :])
                    if out_dmas is not None:
                        out_dmas.append(d)

    nc.compile()
    return nc


def get_nc():
    global _CACHED_NC
    if _CACHED_NC is None:
        _CACHED_NC = _build_nc()
    return _CACHED_NC


def _softmax(x, axis=-1):
    m = np.max(x, axis=axis, keepdims=True)
    e = np.exp(x - m)
    return e / e.sum(axis=axis, keepdims=True)


def _host_routing(x, time_ids, T, gate_w):
    """Replicates _time_aware_summary + video router in float64.

    Returns (expert_idx[B], scores[B, E])."""
    Bb, Nn, Cc = x.shape
    xf = x.astype(np.float64)
    bidx = np.repeat(np.arange(Bb), Nn)
    seg = time_ids.astype(np.int64).reshape(-1) + T * bidx
    token_sum = np.zeros((Bb * T, Cc))
    np.add.at(token_sum, seg, xf.reshape(Bb * Nn, Cc))
    counts = np.bincount(seg, minlength=Bb * T).astype(np.float64)
    token_sum = token_sum.reshape(Bb, T, Cc)
    counts = counts.reshape(Bb, T)
    h_t = token_sum / (counts[..., None] + 1e-6)
    valid = counts > 0
    valid_f = valid.astype(np.float64)[..., None]
    g_app = (h_t * valid_f).sum(1) / np.maximum(valid_f.sum(1), 1.0)
    s_global = np.abs(h_t - g_app[:, None, :]).sum(-1)
    h_prev = np.roll(h_t, 1, axis=1)
    valid_prev = np.roll(valid, 1, axis=1)
    valid_prev[:, 0] = False
    s_diff = np.abs(h_t - h_prev).sum(-1) * (valid & valid_prev).astype(np.float64)
    s = KEY_ALPHA * s_global + (1.0 - KEY_ALPHA) * s_diff
    s = np.where(valid, s, -1e9)
    top_m = max(1, min(KEY_TOP_M, T))
    idx = np.argsort(-s, axis=1, kind="stable")[:, :top_m]
    top_scores = np.take_along_axis(s, idx, axis=1)
    top_w = _softmax(top_scores, axis=-1)
    h_top = np.take_along_axis(h_t, idx[..., None], axis=1)
    g_key = (top_w[..., None] * h_top).sum(1)
    x_video = np.concatenate([g_app, g_key], axis=-1)
    scores = _softmax(x_video @ gate_w.astype(np.float64).T, axis=-1)
    expert_idx = scores.argmax(-1)
    return expert_idx, scores


def _pack_inputs(x, fc1_w, fc1_b, fc2_w, fc2_b, fus_w1, fus_b1, fus_w2,
                 fus_b2, expert_idx):
    f32 = np.float32
    fw1t_full = np.ascontiguousarray(fus_w1[:, :C].T, dtype=f32)
    fw2t_full = np.ascontiguousarray(fus_w2.T, dtype=f32)
    in_maps = []
    for b in range(B):
        e = int(expert_idx[b])
        bias = np.empty((P, NB), dtype=f32)
        bias[:, 0:HC] = fc1_b[e].reshape(HC, P).T
        bias[:, HC:HC + CC] = fc2_b[e].reshape(CC, P).T
        bias[:, HC + CC:HC + 2 * CC] = \
            (fus_b1 + fus_w1[:, C + e]).reshape(CC, P).T
        bias[:, HC + 2 * CC:HC + 3 * CC] = fus_b2.reshape(CC, P).T
        in_maps.append({
            "xt": np.ascontiguousarray(x[b].T, dtype=f32),
            "w1t": np.ascontiguousarray(fc1_w[e].T, dtype=f32),
            "w2t": np.ascontiguousarray(fc2_w[e].T, dtype=f32),
            "fw1t": fw1t_full,
            "fw2t": fw2t_full,
            "bias": bias,
        })
    return in_maps


def kernel(x, time_ids, num_time_bins, gate_w, token_router_w,
           fc1_w, fc1_b, fc2_w, fc2_b, fus_w1, fus_b1, fus_w2, fus_b2):
    x = np.asarray(x)
    time_ids = np.asarray(time_ids)
    T = int(np.asarray(num_time_bins))
    gate_w = np.asarray(gate_w)
    fc1_w = np.asarray(fc1_w)
    fc1_b = np.asarray(fc1_b)
    fc2_w = np.asarray(fc2_w)
    fc2_b = np.asarray(fc2_b)
    fus_w1 = np.asarray(fus_w1)
    fus_b1 = np.asarray(fus_b1)
    fus_w2 = np.asarray(fus_w2)
    fus_b2 = np.asarray(fus_b2)

    expert_idx, scores = _host_routing(x, time_ids, T, gate_w)

    # balance loss: every token of batch b goes to expert e_b
    counts_e = N * np.bincount(expert_idx, minlength=E).astype(np.float64)
    f_e = counts_e / (B * N + 1e-6)
    balance_loss = np.float32(E * (f_e * scores.mean(0)).sum())

    in_maps = _pack_inputs(x, fc1_w, fc1_b, fc2_w, fc2_b, fus_w1, fus_b1,
                           fus_w2, fus_b2, expert_idx)

    nc = get_nc()
    try:
        res = run_bass_kernel_spmd(nc, in_maps, list(range(B)))
    except Exception:
        # transient NRT errors have been observed right after compiles;
        # one retry on a healthy device succeeds
        res = run_bass_kernel_spmd(nc, in_maps, list(range(B)))

    out = np.empty((B, N, C), dtype=np.float32)
    for b in range(B):
        out[b] = res.results[b]["outT"].T
    return out, balance_loss
